# revision 1
# baseline (speedup 1.0000x reference)
"""Trainium2 Bass kernel for the CGProduct problem.

Computes, for each batch element b, the Clebsch-Gordan (CG) product of two
SO(3) irrep feature sets A (l=0..3) and B (l=0..3), tau=16 channels each:

    out[b, M, p, q] (per output l and valid (l1,l2) pair)
        = sum_{m1} CG[M, m1, M-m1] * A_l1[b, m1, p] * B_l2[b, m2, q]

packed per the reference layout: concat over l of [2l+1, npairs_l, 256]
flattened (M-major, pair, then p*16+q), giving [batch, 39936] fp32.

Strategy (8 NeuronCores, batch-parallel 4096 -> 8 x 512):
  * batch on the 128 SBUF partitions, 4 tiles of 128 per core
  * avec/bvec = concatenated A/B rows, [128, 256] each
  * stage 1 (ScalarE): outer-product rows  O_r1[:, p, r2, q] =
        bvec[:, r2, q] * avec[:, r1*16+p]   (activation Copy, per-partition scale)
  * stage 2 (VectorE): per output (l, pair, M) block [128, 256]:
        first term  = tensor_scalar_mul(c * O_strided)
        later terms = scalar_tensor_tensor acc += c * O_strided
  * outputs DMA'd straight to their strided HBM locations (one DMA per
    (l1-group, l) section, 3-D access pattern)
"""

import math
import os
import sys

import numpy as np

for _p in ("/root/.axon_site/_ro/trn_rl_repo",):
    if os.path.isdir(_p) and _p not in sys.path:
        sys.path.insert(0, _p)

import concourse.bass as bass  # noqa: E402
import concourse.tile as tile  # noqa: E402
from concourse import bacc, mybir  # noqa: E402
from concourse.bass_utils import run_bass_kernel_spmd  # noqa: E402

N_CORES = 8
BATCH = 4096
PB = BATCH // N_CORES          # 512 batches per core
P = 128                        # SBUF partitions per tile
NTILES = PB // P               # 4
TAU = 16
LMAX = 3
ROFF = [0, 1, 4, 9]            # global row offset of (l, m=0) in the 16-row stack
F32 = mybir.dt.float32

# ---------------------------------------------------------------------------
# CG coefficient tables (identical math to the reference)
# ---------------------------------------------------------------------------

def _cg_scalar(j1, m1, j2, m2, j, m):
    if m1 + m2 != m:
        return 0.0
    f = math.factorial
    pref = math.sqrt((2 * j + 1) * f(j + j1 - j2) * f(j - j1 + j2)
                     * f(j1 + j2 - j) / f(j1 + j2 + j + 1))
    pref *= math.sqrt(f(j + m) * f(j - m) * f(j1 + m1) * f(j1 - m1)
                      * f(j2 + m2) * f(j2 - m2))
    s = 0.0
    for k in range(0, j1 + j2 - j + 1):
        d = [k, j1 + j2 - j - k, j1 - m1 - k, j2 + m2 - k,
             j - j2 + m1 + k, j - j1 - m2 + k]
        if any(x < 0 for x in d):
            continue
        denom = 1.0
        for x in d:
            denom *= float(f(x))
        s += (-1.0) ** k / denom
    return pref * s


def _cg_tensor(l1, l2, l):
    C = np.zeros((2 * l + 1, 2 * l1 + 1, 2 * l2 + 1), dtype=np.float32)
    for M in range(-l, l + 1):
        for m1 in range(-l1, l1 + 1):
            m2 = M - m1
            if abs(m2) <= l2:
                C[M + l, m1 + l1, m2 + l2] = _cg_scalar(l1, m1, l2, m2, l, M)
    return C


# ---------------------------------------------------------------------------
# Static problem structure
# ---------------------------------------------------------------------------

PAIRS = {l: [(l1, l2) for l1 in range(4) for l2 in range(4)
             if abs(l1 - l2) <= l <= l1 + l2] for l in range(4)}
NPAIRS = {l: len(PAIRS[l]) for l in range(4)}
WIDTH_L = {l: (2 * l + 1) * NPAIRS[l] * 256 for l in range(4)}
OFF_L = {0: 0}
for _l in range(1, 4):
    OFF_L[_l] = OFF_L[_l - 1] + WIDTH_L[_l - 1]
TOTAL_W = OFF_L[3] + WIDTH_L[3]        # 39936


def build_structure():
    """Chains and group layout.

    Returns:
      groups: list over l1 of dict with
         rows: [r1 global rows]
         sections: list of (l, k0, nk, base_cols, l2list)  (ascending l)
         width: total columns of the group tile
      chains: dict (l, k, Mi) -> dict(terms=[(r1, r2, coeff)], l1=..)
    """
    chains = {}
    for l in range(4):
        for k, (l1, l2) in enumerate(PAIRS[l]):
            C = _cg_tensor(l1, l2, l)
            for Mi in range(2 * l + 1):
                terms = []
                for m1i in range(2 * l1 + 1):
                    m2i = Mi - l - (m1i - l1) + l2
                    if 0 <= m2i < 2 * l2 + 1:
                        c = float(C[Mi, m1i, m2i])
                        if c != 0.0:
                            terms.append((ROFF[l1] + m1i, ROFF[l2] + m2i, c))
                assert terms, (l, k, Mi)
                chains[(l, k, Mi)] = dict(terms=terms, l1=l1)

    groups = []
    for l1 in range(4):
        rows = [ROFF[l1] + i for i in range(2 * l1 + 1)]
        sections = []
        base = 0
        for l in range(4):
            l2list = [l2 for (a, l2) in PAIRS[l] if a == l1]
            if not l2list:
                continue
            k0 = PAIRS[l].index((l1, l2list[0]))
            # pairs for fixed l1 are consecutive in k
            for j, l2 in enumerate(l2list):
                assert PAIRS[l][k0 + j] == (l1, l2)
            nk = len(l2list)
            sections.append(dict(l=l, k0=k0, nk=nk, base=base, l2list=l2list))
            base += (2 * l + 1) * nk * 256
        groups.append(dict(l1=l1, rows=rows, sections=sections, width=base))
    return groups, chains


GROUPS, CHAINS = build_structure()
MAX_GROUP_W = max(g["width"] for g in GROUPS)


def chain_dest(chain_key):
    """(base_col_in_group_tile, l) for a chain's [128,256] block."""
    l, k, Mi = chain_key
    l1 = CHAINS[chain_key]["l1"]
    g = GROUPS[l1]
    for s in g["sections"]:
        if s["l"] == l:
            j = k - s["k0"]
            return s["base"] + Mi * s["nk"] * 256 + j * 256
    raise AssertionError(chain_key)


# ---------------------------------------------------------------------------
# Engine assignment (tunable)
# ---------------------------------------------------------------------------
# row ops:   'act' (nc.scalar.mul) or 'dve' (nc.vector.tensor_scalar_mul)
# chain ops: 'dve' or 'gp'
CONFIG = {
    "row_engine": lambda r1, p: "act",
    "chain_engine": lambda key: "dve",
    "o_bufs": 3,
    "group_bufs": 2,
    "vec_bufs": 2,
}


# ---------------------------------------------------------------------------
# Bass program
# ---------------------------------------------------------------------------

def emit_kernel(tc, in_aps, out_ap):
    nc = tc.nc
    cfg = CONFIG

    # schedule: for each r1, the list of (chain_key, term_idx) to emit
    sched = {r1: [] for r1 in range(16)}
    for key, ch in CHAINS.items():
        for ti, (r1, r2, c) in enumerate(ch["terms"]):
            sched[r1].append((key, ti))

    with tc.tile_pool(name="vec", bufs=cfg["vec_bufs"]) as vpool, \
         tc.tile_pool(name="orow", bufs=cfg["o_bufs"]) as opool, \
         tc.tile_pool(name="outg", bufs=cfg["group_bufs"]) as gpool:
        for t in range(NTILES):
            avec = vpool.tile([P, 256], F32, tag="avec")
            bvec = vpool.tile([P, 256], F32, tag="bvec")
            for l in range(4):
                w = (2 * l + 1) * TAU
                src_a = in_aps[l][t * P:(t + 1) * P].rearrange("b m c -> b (m c)")
                src_b = in_aps[4 + l][t * P:(t + 1) * P].rearrange("b m c -> b (m c)")
                nc.sync.dma_start(avec[:, ROFF[l] * TAU: ROFF[l] * TAU + w], src_a)
                nc.sync.dma_start(bvec[:, ROFF[l] * TAU: ROFF[l] * TAU + w], src_b)

            for g in GROUPS:
                gt = gpool.tile([P, g["width"]], F32, tag="outg")

                for r1 in g["rows"]:
                    orow = opool.tile([P, 4096], F32, tag="orow")
                    for p in range(16):
                        eng = cfg["row_engine"](r1, p)
                        dst = orow[:, p * 256:(p + 1) * 256]
                        scal = avec[:, r1 * 16 + p: r1 * 16 + p + 1]
                        if eng == "act":
                            nc.scalar.mul(dst, bvec[:], scal)
                        elif eng == "gp":
                            nc.gpsimd.tensor_scalar_mul(dst, bvec[:], scal)
                        else:
                            nc.vector.tensor_scalar_mul(dst, bvec[:], scal)

                    oview = orow[:].rearrange("B (p m q) -> B p m q", p=16, m=16, q=16)
                    for key, ti in sched[r1]:
                        terms = CHAINS[key]["terms"]
                        _, r2, c = terms[ti]
                        base = chain_dest(key)
                        dst = gt[:, base:base + 256].rearrange(
                            "B (p q) -> B p q", p=16)
                        src = oview[:, :, r2, :]
                        eng = cfg["chain_engine"](key)
                        if ti == 0:
                            if eng == "gp":
                                nc.gpsimd.tensor_scalar_mul(dst, src, c)
                            elif eng == "act":
                                nc.scalar.mul(dst, src, c)
                            else:
                                nc.vector.tensor_scalar_mul(dst, src, c)
                        else:
                            e = nc.gpsimd if eng == "gp" else nc.vector
                            e.scalar_tensor_tensor(
                                dst, src, c, dst,
                                mybir.AluOpType.mult, mybir.AluOpType.add)

                # stream finished group sections to HBM
                for s in g["sections"]:
                    l, k0, nk = s["l"], s["k0"], s["nk"]
                    wsec = (2 * l + 1) * nk * 256
                    src = gt[:, s["base"]: s["base"] + wsec].rearrange(
                        "B (M x) -> B M x", M=2 * l + 1)
                    off = (t * P) * TOTAL_W + OFF_L[l] + k0 * 256
                    dst = bass.AP(out_ap.tensor, off,
                                  [[TOTAL_W, P],
                                   [NPAIRS[l] * 256, 2 * l + 1],
                                   [1, nk * 256]])
                    nc.sync.dma_start(dst, src)


def build_program():
    nc = bacc.Bacc("TRN2", target_bir_lowering=False, debug=False,
                   num_devices=N_CORES)
    in_aps = []
    names = [f"A{l}" for l in range(4)] + [f"B{l}" for l in range(4)]
    for name in names:
        l = int(name[1])
        h = nc.dram_tensor(name, [PB, 2 * l + 1, TAU], F32, kind="ExternalInput")
        in_aps.append(h.ap())
    out_h = nc.dram_tensor("out", [PB, TOTAL_W], F32, kind="ExternalOutput")
    with tile.TileContext(nc) as tc:
        emit_kernel(tc, in_aps, out_h.ap())
    nc.compile()
    return nc


_PROGRAM_CACHE = {}


def get_program():
    if "nc" not in _PROGRAM_CACHE:
        _PROGRAM_CACHE["nc"] = build_program()
    return _PROGRAM_CACHE["nc"]


def kernel(A0, A1, A2, A3, B0, B1, B2, B3, _trace=False):
    ins = dict(A0=A0, A1=A1, A2=A2, A3=A3, B0=B0, B1=B1, B2=B2, B3=B3)
    ins = {k: np.ascontiguousarray(np.asarray(v), dtype=np.float32)
           for k, v in ins.items()}
    nc = get_program()
    in_maps = []
    for core in range(N_CORES):
        sl = slice(core * PB, (core + 1) * PB)
        in_maps.append({k: v[sl] for k, v in ins.items()})
    res = run_bass_kernel_spmd(nc, in_maps, list(range(N_CORES)),
                               trace=_trace)
    out = np.concatenate([np.asarray(r["out"]) for r in res.results], axis=0)
    if _trace:
        kernel.last_exec_time_ns = res.exec_time_ns
        kernel.last_results = res
    return out
